# revision 9
# baseline (speedup 1.0000x reference)
"""Trainium2 Bass kernel for batch-axis-softmax dot-product attention.

Problem: B=8, S=4096, D=64 fp32.
    scores = einsum('bqd,bkd->bqk', Q, K) / 8
    attn   = softmax(scores, axis=0)          # over the BATCH axis!
    out    = einsum('bqk,bkd->bqd', attn, V)

The batch-axis softmax couples only the 8 batch entries of a fixed (q, k)
position, so sharding over the *query* axis (512 queries per core, K/V
replicated) keeps the softmax fully local to each core.

Per-core pipeline, per k-tile (128 keys x 512 queries, all 8 batches):
  PE : scoresT[k,q] = K_tile @ Q^T        (fp16 in, fp32 psum; pairs of
       batches packed into partition halves -> row-tiled concurrent MMs)
  ACT: E_b = exp(0.125 * scoresT_b)       (psum -> sbuf fp16)
  PE : Z = sum_b E_b via identity-matmul accumulation (psum fp32)
  DVE: R = 1/Z (fast approx), cast fp16
  DVE: W_b = E_b * R                      (fp16, 2x mode)
  PE : outT_b[d,q] += V_tile^T-form matmul, accumulated across all 32
       k-tiles in persistent psum (2 batches per bank via column tiling)
Epilogue: DVE copies psum -> sbuf, DMA to HBM; host reassembles.
"""

import numpy as np

B = 8
S = 4096
D = 64
NCORES = 8
QBLK = S // NCORES  # 512 queries per core
KT = 128            # keys per k-tile
NKT = S // KT       # 32 k-tiles
NPAIR = B // 2      # batch pairs packed into 128 partitions

# test.py can flip these before calling kernel()
TRACE = False
TRACE_KWARGS = {}
LAST_RESULT = None  # BassKernelResults of the most recent run (for profiling)

_cache = {}


def _build_nc():
    from contextlib import ExitStack

    import concourse.tile as tile
    from concourse import bacc, mybir

    f16 = mybir.dt.float16
    f32 = mybir.dt.float32
    Exp = mybir.ActivationFunctionType.Exp

    # Bacc (not raw Bass): its finalize() runs the legalization passes that
    # split multi-wait sync_info into EventSemaphore instructions (TRN2 allows
    # at most one wait per regular instruction).
    nc = bacc.Bacc()

    # Inputs pre-arranged on host into exact SBUF layouts (fp16):
    #   qt[p, j*512 + q] = Q[2j + p//64, cblk*512 + q, p%64]
    #   kt[p, j*4096 + k] = K[2j + p//64, k, p%64]
    #   vv[p, b*2048 + n*64 + d] = V[b, n*128 + p, d]
    qt_d = nc.dram_tensor("qt", [128, NPAIR * QBLK], f16, kind="ExternalInput")
    kt_d = nc.dram_tensor("kt", [128, NPAIR * S], f16, kind="ExternalInput")
    vv_d = nc.dram_tensor("vv", [128, B * NKT * D], f16, kind="ExternalInput")
    id_d = nc.dram_tensor("ident", [128, 128], f16, kind="ExternalInput")
    # out[j][(b%2)*64 + d, q] = out_bqd[2j + b%2, q, d]
    out_d = nc.dram_tensor("out", [NPAIR, 128, QBLK], f32, kind="ExternalOutput")

    with tile.TileContext(nc) as tc, ExitStack() as ctx:
        const_p = ctx.enter_context(tc.tile_pool(name="const", bufs=1))
        in_p = ctx.enter_context(tc.tile_pool(name="inp", bufs=1))
        e_p = ctx.enter_context(tc.tile_pool(name="e", bufs=12))
        w_p = ctx.enter_context(tc.tile_pool(name="w", bufs=6))
        r_p = ctx.enter_context(tc.tile_pool(name="r", bufs=2))
        st_p = ctx.enter_context(tc.tile_pool(name="stage", bufs=1))
        ps_s = ctx.enter_context(tc.tile_pool(name="ps_s", bufs=3, space="PSUM"))
        ps_z = ctx.enter_context(tc.tile_pool(name="ps_z", bufs=1, space="PSUM"))
        ps_o = ctx.enter_context(tc.tile_pool(name="ps_o", bufs=1, space="PSUM"))

        ident = const_p.tile([128, 128], f16)
        nc.sync.dma_start(out=ident[:], in_=id_d[:])

        qt = in_p.tile([128, NPAIR * QBLK], f16)
        nc.sync.dma_start(out=qt[:], in_=qt_d[:])
        kt = in_p.tile([128, NPAIR * S], f16)
        nc.sync.dma_start(out=kt[:], in_=kt_d[:])
        vv = in_p.tile([128, B * NKT * D], f16)
        nc.sync.dma_start(out=vv[:], in_=vv_d[:])

        # Persistent output accumulators: bank j holds batches 2j (parts
        # 0:64) and 2j+1 (parts 64:128), accumulated over all 32 k-tiles.
        oacc = [
            ps_o.tile([128, QBLK], f32, tag=f"oacc{j}", name=f"oacc{j}")
            for j in range(NPAIR)
        ]

        for t in range(NKT):
            e_tiles = []
            for b in range(B):
                j, rb = b // 2, (b % 2) * 64
                sc = ps_s.tile([128, QBLK], f32, tag="sc")
                nc.tensor.matmul(
                    out=sc[:],
                    lhsT=kt[rb : rb + 64, j * S + t * KT : j * S + (t + 1) * KT],
                    rhs=qt[rb : rb + 64, j * QBLK : (j + 1) * QBLK],
                    start=True,
                    stop=True,
                    tile_position=(rb, 0),
                )
                e = e_p.tile([128, QBLK], f16, tag="e")
                # E = exp(scores / sqrt(D)); scores*0.125 in [-6, 6] so no
                # max-subtraction is needed and fp16 range is safe.
                nc.scalar.activation(e[:], sc[:], Exp, scale=0.125)
                e_tiles.append(e)

            z = ps_z.tile([128, QBLK], f32, tag="z")
            for b in range(B):
                nc.tensor.matmul(
                    out=z[:],
                    lhsT=ident[:],
                    rhs=e_tiles[b][:],
                    start=(b == 0),
                    stop=(b == B - 1),
                )

            # In-place reciprocal in PSUM (no extra SBUF slot -> no extra
            # wait-sem on the DVE op), then cast to fp16 on ScalarE so the
            # normalize-multiply's two inputs share one wait semaphore.
            nc.vector.reciprocal(out=z[:], in_=z[:])
            r16 = r_p.tile([128, QBLK], f16, tag="r16")
            nc.scalar.copy(out=r16[:], in_=z[:])

            for b in range(B):
                j, rb = b // 2, (b % 2) * 64
                w = w_p.tile([128, QBLK], f16, tag="w")
                nc.vector.tensor_mul(w[:], e_tiles[b][:], r16[:])
                nc.tensor.matmul(
                    out=oacc[j][rb : rb + 64, :],
                    lhsT=vv[:, b * (NKT * D) + t * D : b * (NKT * D) + (t + 1) * D],
                    rhs=w[:],
                    start=(t == 0),
                    stop=(t == NKT - 1),
                    tile_position=(0, rb),
                    skip_group_check=True,
                )

        for j in range(NPAIR):
            st = st_p.tile([128, QBLK], f32, tag=f"st{j}")
            nc.vector.tensor_copy(out=st[:], in_=oacc[j][:])
            nc.sync.dma_start(out=out_d[j], in_=st[:])

    return nc


def _get_nc():
    if "nc" not in _cache:
        nc = _build_nc()
        if not nc.is_finalized():
            # Runs Bacc.compile() legalization (wait splitting, reg alloc).
            nc.finalize()
        _cache["nc"] = nc
    return _cache["nc"]


def _host_prep(queries, keys, values):
    """Cast to fp16 and pre-arrange into the SBUF layouts (see _build_nc)."""
    k16 = np.asarray(keys, dtype=np.float16)
    v16 = np.asarray(values, dtype=np.float16)
    q16 = np.asarray(queries, dtype=np.float16)

    # kt: [8,4096,64] -> [8,64,4096] -> [4,128,4096] -> [128, 4*4096]
    kt = np.ascontiguousarray(
        k16.transpose(0, 2, 1).reshape(NPAIR, 128, S).transpose(1, 0, 2).reshape(128, NPAIR * S)
    )
    # vv: [8,4096,64] -> [8,32,128,64] -> [128,8,32,64] -> [128, 16384]
    vv = np.ascontiguousarray(
        v16.reshape(B, NKT, KT, D).transpose(2, 0, 1, 3).reshape(128, B * NKT * D)
    )
    ident = np.eye(128, dtype=np.float16)

    qts = []
    for c in range(NCORES):
        qc = q16[:, c * QBLK : (c + 1) * QBLK, :]  # [8, 512, 64]
        qt = np.ascontiguousarray(
            qc.transpose(0, 2, 1).reshape(NPAIR, 128, QBLK).transpose(1, 0, 2).reshape(128, NPAIR * QBLK)
        )
        qts.append(qt)
    return qts, kt, vv, ident


def kernel(queries, keys, values):
    global LAST_RESULT
    from concourse.bass_utils import run_bass_kernel_spmd

    queries = np.asarray(queries, dtype=np.float32)
    keys = np.asarray(keys, dtype=np.float32)
    values = np.asarray(values, dtype=np.float32)

    nc = _get_nc()
    qts, kt, vv, ident = _host_prep(queries, keys, values)
    in_maps = [
        {"qt": qts[c], "kt": kt, "vv": vv, "ident": ident} for c in range(NCORES)
    ]

    res = run_bass_kernel_spmd(
        nc,
        in_maps,
        list(range(NCORES)),
        trace=TRACE,
        **TRACE_KWARGS,
    )
    LAST_RESULT = res

    out = np.empty((B, S, D), dtype=np.float32)
    for c in range(NCORES):
        o = res.results[c]["out"]  # [4, 128, 512] = [j, (b%2)*64+d, q]
        out[:, c * QBLK : (c + 1) * QBLK, :] = (
            o.reshape(B, D, QBLK).transpose(0, 2, 1)
        )
    return out


# revision 10
# speedup vs baseline: 1.2334x; 1.2334x over previous
"""Trainium2 Bass kernel for batch-axis-softmax dot-product attention.

Problem: B=8, S=4096, D=64 fp32.
    scores = einsum('bqd,bkd->bqk', Q, K) / 8
    attn   = softmax(scores, axis=0)          # over the BATCH axis!
    out    = einsum('bqk,bkd->bqd', attn, V)

The batch-axis softmax couples only the 8 batch entries of a fixed (q, k)
position, so sharding over the *query* axis (512 queries per core, K/V
replicated) keeps the softmax fully local to each core.

Per-core pipeline, per k-tile (128 keys x 512 queries, all 8 batches):
  PE : scoresT[k,q] = K_tile @ Q^T   (fp16, fp32 psum; batch pairs packed
       into partition halves -> row-tiled concurrent MMs; each pair's two
       512-wide outputs land in one 2-bank psum tile)
  ACT: E_pair = exp(0.125 * scores_pair)  (one 1024-wide op per pair)
  DVE/GPSIMD: Z = sum over the 8 batches (fp16 tree of tensor_adds)
  ACT: R = exp(-ln(Z))  = 1/Z, fp16 out    (ln+exp share one table set)
  DVE: W_b = E_b * R                       (fp16, 2x mode)
  PE : outT_b[d,q] += V_tile matmul, accumulated across all 32 k-tiles in
       persistent psum (2 batches per bank via column tiling)
Epilogue: DVE copies psum -> sbuf, DMA to HBM; host reassembles.
"""

import numpy as np

B = 8
S = 4096
D = 64
NCORES = 8
QBLK = S // NCORES  # 512 queries per core
KT = 128            # keys per k-tile
NKT = S // KT       # 32 k-tiles
NPAIR = B // 2      # batch pairs packed into 128 partitions

# test.py can flip these before calling kernel()
TRACE = False
TRACE_KWARGS = {}
LAST_RESULT = None  # BassKernelResults of the most recent run (for profiling)

_cache = {}


def _build_nc():
    from contextlib import ExitStack

    import concourse.tile as tile
    from concourse import bacc, mybir

    f16 = mybir.dt.float16
    f32 = mybir.dt.float32
    Exp = mybir.ActivationFunctionType.Exp
    Ln = mybir.ActivationFunctionType.Ln

    # Bacc (not raw Bass): its finalize() runs the legalization passes that
    # split multi-wait sync_info into EventSemaphore instructions (TRN2 allows
    # at most one wait per regular instruction).
    nc = bacc.Bacc()

    # Inputs pre-arranged on host into exact SBUF layouts (fp16):
    #   qt[p, j*512 + q] = Q[2j + p//64, cblk*512 + q, p%64]
    #   kt[p, j*4096 + k] = K[2j + p//64, k, p%64]
    #   vv[p, b*2048 + n*64 + d] = V[b, n*128 + p, d]
    qt_d = nc.dram_tensor("qt", [128, NPAIR * QBLK], f16, kind="ExternalInput")
    kt_d = nc.dram_tensor("kt", [128, NPAIR * S], f16, kind="ExternalInput")
    vv_d = nc.dram_tensor("vv", [128, B * NKT * D], f16, kind="ExternalInput")
    # out[j][(b%2)*64 + d, q] = out_bqd[2j + b%2, q, d]
    out_d = nc.dram_tensor("out", [NPAIR, 128, QBLK], f32, kind="ExternalOutput")

    with tile.TileContext(nc) as tc, ExitStack() as ctx:
        in_p = ctx.enter_context(tc.tile_pool(name="inp", bufs=1))
        e_p = ctx.enter_context(tc.tile_pool(name="e", bufs=8))
        w_p = ctx.enter_context(tc.tile_pool(name="w", bufs=6))
        t_p = ctx.enter_context(tc.tile_pool(name="tree", bufs=3))
        r_p = ctx.enter_context(tc.tile_pool(name="r", bufs=2))
        st_p = ctx.enter_context(tc.tile_pool(name="stage", bufs=1))
        ps_s = ctx.enter_context(tc.tile_pool(name="ps_s", bufs=2, space="PSUM"))
        ps_o = ctx.enter_context(tc.tile_pool(name="ps_o", bufs=1, space="PSUM"))

        qt = in_p.tile([128, NPAIR * QBLK], f16)
        nc.sync.dma_start(out=qt[:], in_=qt_d[:])
        kt = in_p.tile([128, NPAIR * S], f16)
        for j in range(NPAIR):
            nc.sync.dma_start(
                out=kt[:, j * S : (j + 1) * S], in_=kt_d[:, j * S : (j + 1) * S]
            )
        vv = in_p.tile([128, B * NKT * D], f16)
        VB = NKT * D  # 2048 per batch
        for j in range(NPAIR):
            nc.sync.dma_start(
                out=vv[:, 2 * j * VB : 2 * (j + 1) * VB],
                in_=vv_d[:, 2 * j * VB : 2 * (j + 1) * VB],
            )

        # Persistent output accumulators: bank j holds batches 2j (parts
        # 0:64) and 2j+1 (parts 64:128), accumulated over all 32 k-tiles.
        oacc = [
            ps_o.tile([128, QBLK], f32, tag=f"oacc{j}", name=f"oacc{j}")
            for j in range(NPAIR)
        ]

        for t in range(NKT):
            # --- scores + exp, one 2-bank pack per batch pair ---
            e_packs = []
            for j in range(NPAIR):
                sc = ps_s.tile([128, 2 * QBLK], f32, tag="sc")
                for m in range(2):  # m=0 -> b=2j (rows 0:64), m=1 -> b=2j+1
                    rb = m * 64
                    nc.tensor.matmul(
                        out=sc[:, m * QBLK : (m + 1) * QBLK],
                        lhsT=kt[rb : rb + 64, j * S + t * KT : j * S + (t + 1) * KT],
                        rhs=qt[rb : rb + 64, j * QBLK : (j + 1) * QBLK],
                        start=True,
                        stop=True,
                        tile_position=(rb, 0),
                    )
                e = e_p.tile([128, 2 * QBLK], f16, tag="e")
                # E = exp(scores / sqrt(D)); scores*0.125 in [-6, 6] so no
                # max-subtraction is needed and fp16 range is safe.
                nc.scalar.activation(e[:], sc[:], Exp, scale=0.125)
                e_packs.append(e)

            # --- Z = sum_b E_b: fp16 adds over pack halves (DVE + GpSimd) ---
            s01 = t_p.tile([128, QBLK], f16, tag="s01")
            nc.gpsimd.tensor_add(
                s01[:], e_packs[0][:, :QBLK], e_packs[0][:, QBLK:]
            )
            s23 = t_p.tile([128, QBLK], f16, tag="s23")
            nc.gpsimd.tensor_add(
                s23[:], e_packs[1][:, :QBLK], e_packs[1][:, QBLK:]
            )
            s45 = t_p.tile([128, QBLK], f16, tag="s45")
            nc.vector.tensor_add(
                s45[:], e_packs[2][:, :QBLK], e_packs[2][:, QBLK:]
            )
            s67 = t_p.tile([128, QBLK], f16, tag="s67")
            nc.vector.tensor_add(
                s67[:], e_packs[3][:, :QBLK], e_packs[3][:, QBLK:]
            )
            s03 = t_p.tile([128, QBLK], f16, tag="s03")
            nc.gpsimd.tensor_add(s03[:], s01[:], s23[:])
            s47 = t_p.tile([128, QBLK], f16, tag="s47")
            nc.vector.tensor_add(s47[:], s45[:], s67[:])
            z = t_p.tile([128, QBLK], f16, tag="z")
            nc.vector.tensor_add(z[:], s03[:], s47[:])

            # --- R = 1/Z via exp(-ln(Z)) on ScalarE (shared table set) ---
            lnz = r_p.tile([128, QBLK], f32, tag="lnz")
            nc.scalar.activation(lnz[:], z[:], Ln)
            r16 = r_p.tile([128, QBLK], f16, tag="r16")
            nc.scalar.activation(r16[:], lnz[:], Exp, scale=-1.0)

            # --- W_b = E_b * R; outT_b[d,q] += V_b[t]^T-form matmul ---
            for b in range(B):
                j, m = b // 2, b % 2
                rb = m * 64
                w = w_p.tile([128, QBLK], f16, tag="w")
                nc.vector.tensor_mul(
                    w[:], e_packs[j][:, m * QBLK : (m + 1) * QBLK], r16[:]
                )
                nc.tensor.matmul(
                    out=oacc[j][rb : rb + 64, :],
                    lhsT=vv[:, b * VB + t * D : b * VB + (t + 1) * D],
                    rhs=w[:],
                    start=(t == 0),
                    stop=(t == NKT - 1),
                    tile_position=(0, rb),
                    skip_group_check=True,
                )

        for j in range(NPAIR):
            st = st_p.tile([128, QBLK], f32, tag=f"st{j}")
            nc.vector.tensor_copy(out=st[:], in_=oacc[j][:])
            nc.sync.dma_start(out=out_d[j], in_=st[:])

    return nc


def _get_nc():
    if "nc" not in _cache:
        nc = _build_nc()
        if not nc.is_finalized():
            # Runs Bacc.compile() legalization (wait splitting, reg alloc).
            nc.finalize()
        _cache["nc"] = nc
    return _cache["nc"]


def _host_prep(queries, keys, values):
    """Cast to fp16 and pre-arrange into the SBUF layouts (see _build_nc)."""
    k16 = np.asarray(keys, dtype=np.float16)
    v16 = np.asarray(values, dtype=np.float16)
    q16 = np.asarray(queries, dtype=np.float16)

    # kt: [8,4096,64] -> [8,64,4096] -> [4,128,4096] -> [128, 4*4096]
    kt = np.ascontiguousarray(
        k16.transpose(0, 2, 1).reshape(NPAIR, 128, S).transpose(1, 0, 2).reshape(128, NPAIR * S)
    )
    # vv: [8,4096,64] -> [8,32,128,64] -> [128,8,32,64] -> [128, 16384]
    vv = np.ascontiguousarray(
        v16.reshape(B, NKT, KT, D).transpose(2, 0, 1, 3).reshape(128, B * NKT * D)
    )

    qts = []
    for c in range(NCORES):
        qc = q16[:, c * QBLK : (c + 1) * QBLK, :]  # [8, 512, 64]
        qt = np.ascontiguousarray(
            qc.transpose(0, 2, 1).reshape(NPAIR, 128, QBLK).transpose(1, 0, 2).reshape(128, NPAIR * QBLK)
        )
        qts.append(qt)
    return qts, kt, vv


def kernel(queries, keys, values):
    global LAST_RESULT
    from concourse.bass_utils import run_bass_kernel_spmd

    queries = np.asarray(queries, dtype=np.float32)
    keys = np.asarray(keys, dtype=np.float32)
    values = np.asarray(values, dtype=np.float32)

    nc = _get_nc()
    qts, kt, vv = _host_prep(queries, keys, values)
    in_maps = [{"qt": qts[c], "kt": kt, "vv": vv} for c in range(NCORES)]

    res = run_bass_kernel_spmd(
        nc,
        in_maps,
        list(range(NCORES)),
        trace=TRACE,
        **TRACE_KWARGS,
    )
    LAST_RESULT = res

    out = np.empty((B, S, D), dtype=np.float32)
    for c in range(NCORES):
        o = res.results[c]["out"]  # [4, 128, 512] = [j, (b%2)*64+d, q]
        out[:, c * QBLK : (c + 1) * QBLK, :] = (
            o.reshape(B, D, QBLK).transpose(0, 2, 1)
        )
    return out


# revision 11
# speedup vs baseline: 1.6142x; 1.3087x over previous
"""Trainium2 Bass kernel for batch-axis-softmax dot-product attention.

Problem: B=8, S=4096, D=64 fp32.
    scores = einsum('bqd,bkd->bqk', Q, K) / 8
    attn   = softmax(scores, axis=0)          # over the BATCH axis!
    out    = einsum('bqk,bkd->bqd', attn, V)

The batch-axis softmax couples only the 8 batch entries of a fixed (q, k)
position, so sharding over the *query* axis (512 queries per core, K/V
replicated) keeps the softmax fully local to each core.

Per-core pipeline, per k-tile (128 keys x 512 queries, all 8 batches):
  PE : scoresT[k,q] = K_tile @ Q^T   (fp16, fp32 psum; batch pairs packed
       into partition halves -> row-tiled concurrent MMs; each pair's two
       512-wide outputs land in one 2-bank psum tile)
  ACT: E_pair = exp(0.125 * scores_pair)  (one 1024-wide op per pair)
  DVE/GPSIMD: Z = sum over the 8 batches (fp16 tree of tensor_adds)
  ACT: R = exp(-ln(Z))  = 1/Z, fp16 out    (ln+exp share one table set)
  DVE: W_b = E_b * R                       (fp16, 2x mode)
  PE : outT_b[d,q] += V_tile matmul, accumulated across all 32 k-tiles in
       persistent psum (2 batches per bank via column tiling)
Epilogue: DVE copies psum -> sbuf, DMA to HBM; host reassembles.
"""

import numpy as np

B = 8
S = 4096
D = 64
NCORES = 8
QBLK = S // NCORES  # 512 queries per core
KT = 128            # keys per k-tile
NKT = S // KT       # 32 k-tiles
NPAIR = B // 2      # batch pairs packed into 128 partitions

# test.py can flip these before calling kernel()
TRACE = False
TRACE_KWARGS = {}
LAST_RESULT = None  # BassKernelResults of the most recent run (for profiling)

_cache = {}


def _build_nc():
    from contextlib import ExitStack

    import concourse.tile as tile
    from concourse import bacc, mybir

    f16 = mybir.dt.float16
    f32 = mybir.dt.float32
    Exp = mybir.ActivationFunctionType.Exp
    Ln = mybir.ActivationFunctionType.Ln

    # Bacc (not raw Bass): its finalize() runs the legalization passes that
    # split multi-wait sync_info into EventSemaphore instructions (TRN2 allows
    # at most one wait per regular instruction).
    #
    # insert_act_table_loads maps each activation func to the first table set
    # containing it, which puts Exp in "exp_and_others" and Ln in
    # "natural_log_exp_and_others" — alternating ACT_TABLE_LOADs every k-tile
    # (~80us of ScalarE). Both funcs live in natural_log_exp_and_others, so
    # restrict Exp/Ln membership to that set: one table load for the whole
    # kernel, hoisted out of the loop.
    class _Bacc(bacc.Bacc):
        def insert_act_table_loads(self):
            from concourse import bass as bass_mod
            from concourse.hw_specs import get_activation_tables

            has_activation = any(
                isinstance(i, mybir.InstActivation)
                for b in self.main_func.blocks
                for i in b.instructions
            )
            if not has_activation:
                return
            combined = "natural_log_exp_and_others"
            tables = []
            for name, fns in get_activation_tables(self.m.arch).items():
                if name != combined:
                    fns = fns - {
                        mybir.ActivationFunctionType.Exp,
                        mybir.ActivationFunctionType.Ln,
                    }
                tables.append((name, fns))
            bass_mod._bass_rust.insert_act_table_loads(self, tables)

    nc = _Bacc()

    # Inputs pre-arranged on host into exact SBUF layouts (fp16):
    #   qt[p, j*512 + q] = Q[2j + p//64, cblk*512 + q, p%64]
    #   kt[p, j*4096 + k] = K[2j + p//64, k, p%64]
    #   vv[p, b*2048 + n*64 + d] = V[b, n*128 + p, d]
    qt_d = nc.dram_tensor("qt", [128, NPAIR * QBLK], f16, kind="ExternalInput")
    kt_d = nc.dram_tensor("kt", [128, NPAIR * S], f16, kind="ExternalInput")
    vv_d = nc.dram_tensor("vv", [128, B * NKT * D], f16, kind="ExternalInput")
    # out[j][(b%2)*64 + d, q] = out_bqd[2j + b%2, q, d]
    out_d = nc.dram_tensor("out", [NPAIR, 128, QBLK], f32, kind="ExternalOutput")

    with tile.TileContext(nc) as tc, ExitStack() as ctx:
        in_p = ctx.enter_context(tc.tile_pool(name="inp", bufs=1))
        e_p = ctx.enter_context(tc.tile_pool(name="e", bufs=8))
        w_p = ctx.enter_context(tc.tile_pool(name="w", bufs=6))
        t_p = ctx.enter_context(tc.tile_pool(name="tree", bufs=3))
        r_p = ctx.enter_context(tc.tile_pool(name="r", bufs=2))
        st_p = ctx.enter_context(tc.tile_pool(name="stage", bufs=1))
        ps_s = ctx.enter_context(tc.tile_pool(name="ps_s", bufs=2, space="PSUM"))
        ps_o = ctx.enter_context(tc.tile_pool(name="ps_o", bufs=1, space="PSUM"))

        qt = in_p.tile([128, NPAIR * QBLK], f16)
        nc.sync.dma_start(out=qt[:], in_=qt_d[:])
        kt = in_p.tile([128, NPAIR * S], f16)
        for j in range(NPAIR):
            nc.sync.dma_start(
                out=kt[:, j * S : (j + 1) * S], in_=kt_d[:, j * S : (j + 1) * S]
            )
        vv = in_p.tile([128, B * NKT * D], f16)
        VB = NKT * D  # 2048 per batch
        for j in range(NPAIR):
            nc.sync.dma_start(
                out=vv[:, 2 * j * VB : 2 * (j + 1) * VB],
                in_=vv_d[:, 2 * j * VB : 2 * (j + 1) * VB],
            )

        # Persistent output accumulators: bank j holds batches 2j (parts
        # 0:64) and 2j+1 (parts 64:128), accumulated over all 32 k-tiles.
        oacc = [
            ps_o.tile([128, QBLK], f32, tag=f"oacc{j}", name=f"oacc{j}")
            for j in range(NPAIR)
        ]

        for t in range(NKT):
            # --- scores + exp, one 2-bank pack per batch pair ---
            e_packs = []
            for j in range(NPAIR):
                sc = ps_s.tile([128, 2 * QBLK], f32, tag="sc")
                for m in range(2):  # m=0 -> b=2j (rows 0:64), m=1 -> b=2j+1
                    rb = m * 64
                    nc.tensor.matmul(
                        out=sc[:, m * QBLK : (m + 1) * QBLK],
                        lhsT=kt[rb : rb + 64, j * S + t * KT : j * S + (t + 1) * KT],
                        rhs=qt[rb : rb + 64, j * QBLK : (j + 1) * QBLK],
                        start=True,
                        stop=True,
                        tile_position=(rb, 0),
                    )
                e = e_p.tile([128, 2 * QBLK], f16, tag="e")
                # E = exp(scores / sqrt(D)); scores*0.125 in [-6, 6] so no
                # max-subtraction is needed and fp16 range is safe.
                nc.scalar.activation(e[:], sc[:], Exp, scale=0.125)
                e_packs.append(e)

            # --- Z = sum_b E_b: fp16 adds over pack halves (DVE + GpSimd) ---
            s01 = t_p.tile([128, QBLK], f16, tag="s01")
            nc.gpsimd.tensor_add(
                s01[:], e_packs[0][:, :QBLK], e_packs[0][:, QBLK:]
            )
            s23 = t_p.tile([128, QBLK], f16, tag="s23")
            nc.gpsimd.tensor_add(
                s23[:], e_packs[1][:, :QBLK], e_packs[1][:, QBLK:]
            )
            s45 = t_p.tile([128, QBLK], f16, tag="s45")
            nc.vector.tensor_add(
                s45[:], e_packs[2][:, :QBLK], e_packs[2][:, QBLK:]
            )
            s67 = t_p.tile([128, QBLK], f16, tag="s67")
            nc.vector.tensor_add(
                s67[:], e_packs[3][:, :QBLK], e_packs[3][:, QBLK:]
            )
            s03 = t_p.tile([128, QBLK], f16, tag="s03")
            nc.gpsimd.tensor_add(s03[:], s01[:], s23[:])
            s47 = t_p.tile([128, QBLK], f16, tag="s47")
            nc.vector.tensor_add(s47[:], s45[:], s67[:])
            z = t_p.tile([128, QBLK], f16, tag="z")
            nc.vector.tensor_add(z[:], s03[:], s47[:])

            # --- R = 1/Z via exp(-ln(Z)) on ScalarE (shared table set) ---
            lnz = r_p.tile([128, QBLK], f32, tag="lnz")
            nc.scalar.activation(lnz[:], z[:], Ln)
            r16 = r_p.tile([128, QBLK], f16, tag="r16")
            nc.scalar.activation(r16[:], lnz[:], Exp, scale=-1.0)

            # --- W_b = E_b * R; outT_b[d,q] += V_b[t]^T-form matmul ---
            for b in range(B):
                j, m = b // 2, b % 2
                rb = m * 64
                w = w_p.tile([128, QBLK], f16, tag="w")
                nc.vector.tensor_mul(
                    w[:], e_packs[j][:, m * QBLK : (m + 1) * QBLK], r16[:]
                )
                nc.tensor.matmul(
                    out=oacc[j][rb : rb + 64, :],
                    lhsT=vv[:, b * VB + t * D : b * VB + (t + 1) * D],
                    rhs=w[:],
                    start=(t == 0),
                    stop=(t == NKT - 1),
                    tile_position=(0, rb),
                    skip_group_check=True,
                )

        for j in range(NPAIR):
            st = st_p.tile([128, QBLK], f32, tag=f"st{j}")
            nc.vector.tensor_copy(out=st[:], in_=oacc[j][:])
            nc.sync.dma_start(out=out_d[j], in_=st[:])

    return nc


def _get_nc():
    if "nc" not in _cache:
        nc = _build_nc()
        if not nc.is_finalized():
            # Runs Bacc.compile() legalization (wait splitting, reg alloc).
            nc.finalize()
        _cache["nc"] = nc
    return _cache["nc"]


def _host_prep(queries, keys, values):
    """Cast to fp16 and pre-arrange into the SBUF layouts (see _build_nc)."""
    k16 = np.asarray(keys, dtype=np.float16)
    v16 = np.asarray(values, dtype=np.float16)
    q16 = np.asarray(queries, dtype=np.float16)

    # kt: [8,4096,64] -> [8,64,4096] -> [4,128,4096] -> [128, 4*4096]
    kt = np.ascontiguousarray(
        k16.transpose(0, 2, 1).reshape(NPAIR, 128, S).transpose(1, 0, 2).reshape(128, NPAIR * S)
    )
    # vv: [8,4096,64] -> [8,32,128,64] -> [128,8,32,64] -> [128, 16384]
    vv = np.ascontiguousarray(
        v16.reshape(B, NKT, KT, D).transpose(2, 0, 1, 3).reshape(128, B * NKT * D)
    )

    qts = []
    for c in range(NCORES):
        qc = q16[:, c * QBLK : (c + 1) * QBLK, :]  # [8, 512, 64]
        qt = np.ascontiguousarray(
            qc.transpose(0, 2, 1).reshape(NPAIR, 128, QBLK).transpose(1, 0, 2).reshape(128, NPAIR * QBLK)
        )
        qts.append(qt)
    return qts, kt, vv


def kernel(queries, keys, values):
    global LAST_RESULT
    from concourse.bass_utils import run_bass_kernel_spmd

    queries = np.asarray(queries, dtype=np.float32)
    keys = np.asarray(keys, dtype=np.float32)
    values = np.asarray(values, dtype=np.float32)

    nc = _get_nc()
    qts, kt, vv = _host_prep(queries, keys, values)
    in_maps = [{"qt": qts[c], "kt": kt, "vv": vv} for c in range(NCORES)]

    res = run_bass_kernel_spmd(
        nc,
        in_maps,
        list(range(NCORES)),
        trace=TRACE,
        **TRACE_KWARGS,
    )
    LAST_RESULT = res

    out = np.empty((B, S, D), dtype=np.float32)
    for c in range(NCORES):
        o = res.results[c]["out"]  # [4, 128, 512] = [j, (b%2)*64+d, q]
        out[:, c * QBLK : (c + 1) * QBLK, :] = (
            o.reshape(B, D, QBLK).transpose(0, 2, 1)
        )
    return out


# revision 12
# speedup vs baseline: 1.6290x; 1.0092x over previous
"""Trainium2 Bass kernel for batch-axis-softmax dot-product attention.

Problem: B=8, S=4096, D=64 fp32.
    scores = einsum('bqd,bkd->bqk', Q, K) / 8
    attn   = softmax(scores, axis=0)          # over the BATCH axis!
    out    = einsum('bqk,bkd->bqd', attn, V)

The batch-axis softmax couples only the 8 batch entries of a fixed (q, k)
position, so sharding over the *query* axis (512 queries per core, K/V
replicated) keeps the softmax fully local to each core.

Per-core pipeline, per k-tile (128 keys x 512 queries, all 8 batches):
  PE : scoresT[k,q] = K_tile @ Q^T   (fp16, fp32 psum; batch pairs packed
       into partition halves -> row-tiled concurrent MMs; each pair's two
       512-wide outputs land in one 2-bank psum tile)
  ACT: E_pair = exp(0.125 * scores_pair)  (one 1024-wide op per pair)
  DVE/GPSIMD: Z = sum over the 8 batches (fp16 tree of tensor_adds)
  ACT: R = exp(-ln(Z))  = 1/Z, fp16 out    (ln+exp share one table set)
  DVE: W_b = E_b * R                       (fp16, 2x mode)
  PE : outT_b[d,q] += V_tile matmul, accumulated across all 32 k-tiles in
       persistent psum (2 batches per bank via column tiling)
Epilogue: DVE copies psum -> sbuf, DMA to HBM; host reassembles.
"""

import numpy as np

B = 8
S = 4096
D = 64
NCORES = 8
QBLK = S // NCORES  # 512 queries per core
KT = 128            # keys per k-tile
NKT = S // KT       # 32 k-tiles
NPAIR = B // 2      # batch pairs packed into 128 partitions

# test.py can flip these before calling kernel()
TRACE = False
TRACE_KWARGS = {}
LAST_RESULT = None  # BassKernelResults of the most recent run (for profiling)

_cache = {}


def _build_nc():
    from contextlib import ExitStack

    import concourse.tile as tile
    from concourse import bacc, mybir

    f16 = mybir.dt.float16
    f32 = mybir.dt.float32
    Exp = mybir.ActivationFunctionType.Exp
    Ln = mybir.ActivationFunctionType.Ln

    # Bacc (not raw Bass): its finalize() runs the legalization passes that
    # split multi-wait sync_info into EventSemaphore instructions (TRN2 allows
    # at most one wait per regular instruction).
    #
    # insert_act_table_loads maps each activation func to the first table set
    # containing it, which puts Exp in "exp_and_others" and Ln in
    # "natural_log_exp_and_others" — alternating ACT_TABLE_LOADs every k-tile
    # (~80us of ScalarE). Both funcs live in natural_log_exp_and_others, so
    # restrict Exp/Ln membership to that set: one table load for the whole
    # kernel, hoisted out of the loop.
    class _Bacc(bacc.Bacc):
        def insert_act_table_loads(self):
            from concourse import bass as bass_mod
            from concourse.hw_specs import get_activation_tables

            has_activation = any(
                isinstance(i, mybir.InstActivation)
                for b in self.main_func.blocks
                for i in b.instructions
            )
            if not has_activation:
                return
            combined = "natural_log_exp_and_others"
            tables = []
            for name, fns in get_activation_tables(self.m.arch).items():
                if name != combined:
                    fns = fns - {
                        mybir.ActivationFunctionType.Exp,
                        mybir.ActivationFunctionType.Ln,
                    }
                tables.append((name, fns))
            bass_mod._bass_rust.insert_act_table_loads(self, tables)

    nc = _Bacc()

    # Inputs pre-arranged on host into exact SBUF layouts (fp16):
    #   qt[p, j*512 + q] = Q[2j + p//64, cblk*512 + q, p%64]
    #   kt[p, j*4096 + k] = K[2j + p//64, k, p%64]
    #   vv[p, b*2048 + n*64 + d] = V[b, n*128 + p, d]
    qt_d = nc.dram_tensor("qt", [128, NPAIR * QBLK], f16, kind="ExternalInput")
    kt_d = nc.dram_tensor("kt", [128, NPAIR * S], f16, kind="ExternalInput")
    vv_d = nc.dram_tensor("vv", [128, B * NKT * D], f16, kind="ExternalInput")
    # out[j][(b%2)*64 + d, q] = out_bqd[2j + b%2, q, d]
    out_d = nc.dram_tensor("out", [NPAIR, 128, QBLK], f32, kind="ExternalOutput")

    with tile.TileContext(nc) as tc, ExitStack() as ctx:
        in_p = ctx.enter_context(tc.tile_pool(name="inp", bufs=1))
        e_p = ctx.enter_context(tc.tile_pool(name="e", bufs=8))
        w_p = ctx.enter_context(tc.tile_pool(name="w", bufs=6))
        t_p = ctx.enter_context(tc.tile_pool(name="tree", bufs=3))
        r_p = ctx.enter_context(tc.tile_pool(name="r", bufs=2))
        st_p = ctx.enter_context(tc.tile_pool(name="stage", bufs=1))
        ps_s = ctx.enter_context(tc.tile_pool(name="ps_s", bufs=2, space="PSUM"))
        ps_o = ctx.enter_context(tc.tile_pool(name="ps_o", bufs=1, space="PSUM"))

        qt = in_p.tile([128, NPAIR * QBLK], f16)
        nc.sync.dma_start(out=qt[:], in_=qt_d[:])
        kt = in_p.tile([128, NPAIR * S], f16)
        for j in range(NPAIR):
            nc.sync.dma_start(
                out=kt[:, j * S : (j + 1) * S], in_=kt_d[:, j * S : (j + 1) * S]
            )
        vv = in_p.tile([128, B * NKT * D], f16)
        VB = NKT * D  # 2048 per batch
        for j in range(NPAIR):
            nc.sync.dma_start(
                out=vv[:, 2 * j * VB : 2 * (j + 1) * VB],
                in_=vv_d[:, 2 * j * VB : 2 * (j + 1) * VB],
            )

        # Persistent output accumulators: bank j holds batches 2j (parts
        # 0:64) and 2j+1 (parts 64:128), accumulated over all 32 k-tiles.
        oacc = [
            ps_o.tile([128, QBLK], f32, tag=f"oacc{j}", name=f"oacc{j}")
            for j in range(NPAIR)
        ]

        def emit_scores_exp(t):
            # scores + exp, one 2-bank pack per batch pair
            e_packs = []
            for j in range(NPAIR):
                sc = ps_s.tile([128, 2 * QBLK], f32, tag="sc", name=f"sc{t}_{j}")
                for m in range(2):  # m=0 -> b=2j (rows 0:64), m=1 -> b=2j+1
                    rb = m * 64
                    nc.tensor.matmul(
                        out=sc[:, m * QBLK : (m + 1) * QBLK],
                        lhsT=kt[rb : rb + 64, j * S + t * KT : j * S + (t + 1) * KT],
                        rhs=qt[rb : rb + 64, j * QBLK : (j + 1) * QBLK],
                        start=True,
                        stop=True,
                        tile_position=(rb, 0),
                    )
                e = e_p.tile([128, 2 * QBLK], f16, tag="e", name=f"e{t}_{j}")
                # E = exp(scores / sqrt(D)); scores*0.125 in [-6, 6] so no
                # max-subtraction is needed and fp16 range is safe.
                nc.scalar.activation(e[:], sc[:], Exp, scale=0.125)
                e_packs.append(e)
            return e_packs

        def emit_tree_r(t, e_packs):
            # Z = sum_b E_b: fp16 adds over pack halves (DVE + GpSimd)
            s01 = t_p.tile([128, QBLK], f16, tag="s01", name=f"s01_{t}")
            nc.gpsimd.tensor_add(
                s01[:], e_packs[0][:, :QBLK], e_packs[0][:, QBLK:]
            )
            s23 = t_p.tile([128, QBLK], f16, tag="s23", name=f"s23_{t}")
            nc.gpsimd.tensor_add(
                s23[:], e_packs[1][:, :QBLK], e_packs[1][:, QBLK:]
            )
            s45 = t_p.tile([128, QBLK], f16, tag="s45", name=f"s45_{t}")
            nc.vector.tensor_add(
                s45[:], e_packs[2][:, :QBLK], e_packs[2][:, QBLK:]
            )
            s67 = t_p.tile([128, QBLK], f16, tag="s67", name=f"s67_{t}")
            nc.vector.tensor_add(
                s67[:], e_packs[3][:, :QBLK], e_packs[3][:, QBLK:]
            )
            s03 = t_p.tile([128, QBLK], f16, tag="s03", name=f"s03_{t}")
            nc.gpsimd.tensor_add(s03[:], s01[:], s23[:])
            s47 = t_p.tile([128, QBLK], f16, tag="s47", name=f"s47_{t}")
            nc.vector.tensor_add(s47[:], s45[:], s67[:])
            z = t_p.tile([128, QBLK], f16, tag="z", name=f"z{t}")
            nc.vector.tensor_add(z[:], s03[:], s47[:])

            # R = 1/Z via exp(-ln(Z)) on ScalarE (shared table set)
            lnz = r_p.tile([128, QBLK], f32, tag="lnz", name=f"lnz{t}")
            nc.scalar.activation(lnz[:], z[:], Ln)
            r16 = r_p.tile([128, QBLK], f16, tag="r16", name=f"r16_{t}")
            nc.scalar.activation(r16[:], lnz[:], Exp, scale=-1.0)
            return r16

        def emit_mults_av(t, e_packs, r16):
            # W_b = E_b * R; outT_b[d,q] += V_b[t]^T-form matmul
            for b in range(B):
                j, m = b // 2, b % 2
                rb = m * 64
                w = w_p.tile([128, QBLK], f16, tag="w", name=f"w{t}_{b}")
                nc.vector.tensor_mul(
                    w[:], e_packs[j][:, m * QBLK : (m + 1) * QBLK], r16[:]
                )
                nc.tensor.matmul(
                    out=oacc[j][rb : rb + 64, :],
                    lhsT=vv[:, b * VB + t * D : b * VB + (t + 1) * D],
                    rhs=w[:],
                    start=(t == 0),
                    stop=(t == NKT - 1),
                    tile_position=(0, rb),
                    skip_group_check=True,
                )

        # Software-pipelined by one k-tile: scores(t+1) are emitted before
        # mults/AV(t) so the strict-FIFO PE queue never stalls on the softmax
        # chain (sc(t+1) MMs issue while ACT/DVE work on tile t; AV(t) is
        # ready by the time it reaches the head of the queue).
        prev = None
        for t in range(NKT):
            e_packs = emit_scores_exp(t)
            if prev is not None:
                emit_mults_av(*prev)
            r16 = emit_tree_r(t, e_packs)
            prev = (t, e_packs, r16)
        emit_mults_av(*prev)

        for j in range(NPAIR):
            st = st_p.tile([128, QBLK], f32, tag=f"st{j}")
            nc.vector.tensor_copy(out=st[:], in_=oacc[j][:])
            nc.sync.dma_start(out=out_d[j], in_=st[:])

    return nc


def _get_nc():
    if "nc" not in _cache:
        nc = _build_nc()
        if not nc.is_finalized():
            # Runs Bacc.compile() legalization (wait splitting, reg alloc).
            nc.finalize()
        _cache["nc"] = nc
    return _cache["nc"]


def _host_prep(queries, keys, values):
    """Cast to fp16 and pre-arrange into the SBUF layouts (see _build_nc)."""
    k16 = np.asarray(keys, dtype=np.float16)
    v16 = np.asarray(values, dtype=np.float16)
    q16 = np.asarray(queries, dtype=np.float16)

    # kt: [8,4096,64] -> [8,64,4096] -> [4,128,4096] -> [128, 4*4096]
    kt = np.ascontiguousarray(
        k16.transpose(0, 2, 1).reshape(NPAIR, 128, S).transpose(1, 0, 2).reshape(128, NPAIR * S)
    )
    # vv: [8,4096,64] -> [8,32,128,64] -> [128,8,32,64] -> [128, 16384]
    vv = np.ascontiguousarray(
        v16.reshape(B, NKT, KT, D).transpose(2, 0, 1, 3).reshape(128, B * NKT * D)
    )

    qts = []
    for c in range(NCORES):
        qc = q16[:, c * QBLK : (c + 1) * QBLK, :]  # [8, 512, 64]
        qt = np.ascontiguousarray(
            qc.transpose(0, 2, 1).reshape(NPAIR, 128, QBLK).transpose(1, 0, 2).reshape(128, NPAIR * QBLK)
        )
        qts.append(qt)
    return qts, kt, vv


def kernel(queries, keys, values):
    global LAST_RESULT
    from concourse.bass_utils import run_bass_kernel_spmd

    queries = np.asarray(queries, dtype=np.float32)
    keys = np.asarray(keys, dtype=np.float32)
    values = np.asarray(values, dtype=np.float32)

    nc = _get_nc()
    qts, kt, vv = _host_prep(queries, keys, values)
    in_maps = [{"qt": qts[c], "kt": kt, "vv": vv} for c in range(NCORES)]

    res = run_bass_kernel_spmd(
        nc,
        in_maps,
        list(range(NCORES)),
        trace=TRACE,
        **TRACE_KWARGS,
    )
    LAST_RESULT = res

    out = np.empty((B, S, D), dtype=np.float32)
    for c in range(NCORES):
        o = res.results[c]["out"]  # [4, 128, 512] = [j, (b%2)*64+d, q]
        out[:, c * QBLK : (c + 1) * QBLK, :] = (
            o.reshape(B, D, QBLK).transpose(0, 2, 1)
        )
    return out


# revision 14
# speedup vs baseline: 1.6551x; 1.0160x over previous
"""Trainium2 Bass kernel for batch-axis-softmax dot-product attention.

Problem: B=8, S=4096, D=64 fp32.
    scores = einsum('bqd,bkd->bqk', Q, K) / 8
    attn   = softmax(scores, axis=0)          # over the BATCH axis!
    out    = einsum('bqk,bkd->bqd', attn, V)

The batch-axis softmax couples only the 8 batch entries of a fixed (q, k)
position, so sharding over the *query* axis (512 queries per core, K/V
replicated) keeps the softmax fully local to each core.

Per-core pipeline, per k-tile (128 keys x 512 queries, all 8 batches):
  PE : scoresT[k,q] = K_tile @ Q^T   (fp16, fp32 psum; batch pairs packed
       into partition halves -> row-tiled concurrent MMs; each pair's two
       512-wide outputs land in one 2-bank psum tile)
  ACT: E_pair = exp(0.125 * scores_pair)  (one 1024-wide op per pair)
  DVE/GPSIMD: Z = sum over the 8 batches (fp16 tree of tensor_adds)
  ACT: R = exp(-ln(Z))  = 1/Z, fp16 out    (ln+exp share one table set)
  DVE: W_b = E_b * R                       (fp16, 2x mode)
  PE : outT_b[d,q] += V_tile matmul, accumulated across all 32 k-tiles in
       persistent psum (2 batches per bank via column tiling)
Epilogue: DVE copies psum -> sbuf, DMA to HBM; host reassembles.
"""

import numpy as np

B = 8
S = 4096
D = 64
NCORES = 8
QBLK = S // NCORES  # 512 queries per core
KT = 128            # keys per k-tile
NKT = S // KT       # 32 k-tiles
NPAIR = B // 2      # batch pairs packed into 128 partitions

# test.py can flip these before calling kernel()
TRACE = False
TRACE_KWARGS = {}
LAST_RESULT = None  # BassKernelResults of the most recent run (for profiling)

_cache = {}


def _build_nc():
    from contextlib import ExitStack

    import concourse.tile as tile
    from concourse import bacc, mybir

    f16 = mybir.dt.float16
    f32 = mybir.dt.float32
    Exp = mybir.ActivationFunctionType.Exp
    Ln = mybir.ActivationFunctionType.Ln

    # Bacc (not raw Bass): its finalize() runs the legalization passes that
    # split multi-wait sync_info into EventSemaphore instructions (TRN2 allows
    # at most one wait per regular instruction).
    #
    # insert_act_table_loads maps each activation func to the first table set
    # containing it, which puts Exp in "exp_and_others" and Ln in
    # "natural_log_exp_and_others" — alternating ACT_TABLE_LOADs every k-tile
    # (~80us of ScalarE). Both funcs live in natural_log_exp_and_others, so
    # restrict Exp/Ln membership to that set: one table load for the whole
    # kernel, hoisted out of the loop.
    class _Bacc(bacc.Bacc):
        def insert_act_table_loads(self):
            from concourse import bass as bass_mod
            from concourse.hw_specs import get_activation_tables

            has_activation = any(
                isinstance(i, mybir.InstActivation)
                for b in self.main_func.blocks
                for i in b.instructions
            )
            if not has_activation:
                return
            combined = "natural_log_exp_and_others"
            tables = []
            for name, fns in get_activation_tables(self.m.arch).items():
                if name != combined:
                    fns = fns - {
                        mybir.ActivationFunctionType.Exp,
                        mybir.ActivationFunctionType.Ln,
                    }
                tables.append((name, fns))
            bass_mod._bass_rust.insert_act_table_loads(self, tables)

    nc = _Bacc()

    # Inputs pre-arranged on host into exact SBUF layouts (fp16):
    #   qt[p, j*512 + q] = Q[2j + p//64, cblk*512 + q, p%64]
    #   kt[p, j*4096 + k] = K[2j + p//64, k, p%64]
    #   vv[p, b*2048 + n*64 + d] = V[b, n*128 + p, d]
    qt_d = nc.dram_tensor("qt", [128, NPAIR * QBLK], f16, kind="ExternalInput")
    kt_d = nc.dram_tensor("kt", [128, NPAIR * S], f16, kind="ExternalInput")
    vv_d = nc.dram_tensor("vv", [128, B * NKT * D], f16, kind="ExternalInput")
    # out[j][(b%2)*64 + d, q] = out_bqd[2j + b%2, q, d]
    out_d = nc.dram_tensor("out", [NPAIR, 128, QBLK], f32, kind="ExternalOutput")

    with tile.TileContext(nc) as tc, ExitStack() as ctx:
        in_p = ctx.enter_context(tc.tile_pool(name="inp", bufs=1))
        e_p = ctx.enter_context(tc.tile_pool(name="e", bufs=8))
        w_p = ctx.enter_context(tc.tile_pool(name="w", bufs=6))
        t_p = ctx.enter_context(tc.tile_pool(name="tree", bufs=3))
        r_p = ctx.enter_context(tc.tile_pool(name="r", bufs=2))
        st_p = ctx.enter_context(tc.tile_pool(name="stage", bufs=1))
        ps_s = ctx.enter_context(tc.tile_pool(name="ps_s", bufs=2, space="PSUM"))
        ps_o = ctx.enter_context(tc.tile_pool(name="ps_o", bufs=1, space="PSUM"))

        qt = in_p.tile([128, NPAIR * QBLK], f16)
        nc.sync.dma_start(out=qt[:], in_=qt_d[:])
        kt = in_p.tile([128, NPAIR * S], f16)
        for j in range(NPAIR):
            nc.sync.dma_start(
                out=kt[:, j * S : (j + 1) * S], in_=kt_d[:, j * S : (j + 1) * S]
            )
        vv = in_p.tile([128, B * NKT * D], f16)
        VB = NKT * D  # 2048 per batch
        for j in range(NPAIR):
            nc.sync.dma_start(
                out=vv[:, 2 * j * VB : 2 * (j + 1) * VB],
                in_=vv_d[:, 2 * j * VB : 2 * (j + 1) * VB],
            )

        # Persistent output accumulators: bank j holds batches 2j (parts
        # 0:64) and 2j+1 (parts 64:128), accumulated over all 32 k-tiles.
        oacc = [
            ps_o.tile([128, QBLK], f32, tag=f"oacc{j}", name=f"oacc{j}")
            for j in range(NPAIR)
        ]

        def emit_scores_exp(t):
            # scores + exp, one 2-bank pack per batch pair
            e_packs = []
            for j in range(NPAIR):
                sc = ps_s.tile([128, 2 * QBLK], f32, tag="sc", name=f"sc{t}_{j}")
                for m in range(2):  # m=0 -> b=2j (rows 0:64), m=1 -> b=2j+1
                    rb = m * 64
                    nc.tensor.matmul(
                        out=sc[:, m * QBLK : (m + 1) * QBLK],
                        lhsT=kt[rb : rb + 64, j * S + t * KT : j * S + (t + 1) * KT],
                        rhs=qt[rb : rb + 64, j * QBLK : (j + 1) * QBLK],
                        start=True,
                        stop=True,
                        tile_position=(rb, 0),
                    )
                e = e_p.tile([128, 2 * QBLK], f16, tag="e", name=f"e{t}_{j}")
                # E = exp(scores / sqrt(D)); scores*0.125 in [-6, 6] so no
                # max-subtraction is needed and fp16 range is safe.
                nc.scalar.activation(e[:], sc[:], Exp, scale=0.125)
                e_packs.append(e)
            return e_packs

        def emit_tree_r(t, e_packs):
            # Z = sum_b E_b: fp16 adds over pack halves (DVE + GpSimd)
            s01 = t_p.tile([128, QBLK], f16, tag="s01", name=f"s01_{t}")
            nc.gpsimd.tensor_add(
                s01[:], e_packs[0][:, :QBLK], e_packs[0][:, QBLK:]
            )
            s23 = t_p.tile([128, QBLK], f16, tag="s23", name=f"s23_{t}")
            nc.gpsimd.tensor_add(
                s23[:], e_packs[1][:, :QBLK], e_packs[1][:, QBLK:]
            )
            s45 = t_p.tile([128, QBLK], f16, tag="s45", name=f"s45_{t}")
            nc.vector.tensor_add(
                s45[:], e_packs[2][:, :QBLK], e_packs[2][:, QBLK:]
            )
            s67 = t_p.tile([128, QBLK], f16, tag="s67", name=f"s67_{t}")
            nc.vector.tensor_add(
                s67[:], e_packs[3][:, :QBLK], e_packs[3][:, QBLK:]
            )
            s03 = t_p.tile([128, QBLK], f16, tag="s03", name=f"s03_{t}")
            nc.gpsimd.tensor_add(s03[:], s01[:], s23[:])
            s47 = t_p.tile([128, QBLK], f16, tag="s47", name=f"s47_{t}")
            nc.vector.tensor_add(s47[:], s45[:], s67[:])
            z = t_p.tile([128, QBLK], f16, tag="z", name=f"z{t}")
            nc.vector.tensor_add(z[:], s03[:], s47[:])

            # R = 1/Z via exp(-ln(Z)) on ScalarE (shared table set)
            lnz = r_p.tile([128, QBLK], f32, tag="lnz", name=f"lnz{t}")
            nc.scalar.activation(lnz[:], z[:], Ln)
            r16 = r_p.tile([128, QBLK], f16, tag="r16", name=f"r16_{t}")
            nc.scalar.activation(r16[:], lnz[:], Exp, scale=-1.0)
            return r16

        def emit_mults_av(t, e_packs, r16):
            # W_b = E_b * R; outT_b[d,q] += V_b[t]^T-form matmul.
            # All 8 mults are emitted before any AV matmul, and the AV
            # matmuls consume them in REVERSE order: the first AV's wait (on
            # the last mult's DVE tick) covers all the rest, so AVs 2..8
            # issue back-to-back with no intervening semaphore stalls.
            ws = []
            for b in range(B):
                j, m = b // 2, b % 2
                w = w_p.tile([128, QBLK], f16, tag="w", name=f"w{t}_{b}")
                nc.vector.tensor_mul(
                    w[:], e_packs[j][:, m * QBLK : (m + 1) * QBLK], r16[:]
                )
                ws.append(w)
            for b in reversed(range(B)):
                j, m = b // 2, b % 2
                rb = m * 64
                nc.tensor.matmul(
                    out=oacc[j][rb : rb + 64, :],
                    lhsT=vv[:, b * VB + t * D : b * VB + (t + 1) * D],
                    rhs=ws[b][:],
                    start=(t == 0),
                    stop=(t == NKT - 1),
                    tile_position=(0, rb),
                    skip_group_check=True,
                )

        # HAM warmup: ~5us of back-to-back matmuls into a scratch psum tile
        # trips the PE clock gate from 1.2 GHz to 2.4 GHz before the main
        # loop (the loop's own bursts are too fragmented to ever warm it).
        warm = ps_s.tile([128, 2 * QBLK], f32, tag="sc", name="warmup")
        for i in range(12):
            nc.tensor.matmul(
                out=warm[:, :QBLK],
                lhsT=kt[0:64, 0:KT],
                rhs=qt[0:64, 0:QBLK],
                start=True,
                stop=True,
                tile_position=(0, 0),
            )

        # Software-pipelined by one k-tile: scores(t+1) are emitted before
        # mults/AV(t) so the strict-FIFO PE queue never stalls on the softmax
        # chain (sc(t+1) MMs issue while ACT/DVE work on tile t; AV(t) is
        # ready by the time it reaches the head of the queue).
        prev = None
        for t in range(NKT):
            e_packs = emit_scores_exp(t)
            if prev is not None:
                emit_mults_av(*prev)
            r16 = emit_tree_r(t, e_packs)
            prev = (t, e_packs, r16)
        emit_mults_av(*prev)

        for j in range(NPAIR):
            st = st_p.tile([128, QBLK], f32, tag=f"st{j}")
            nc.vector.tensor_copy(out=st[:], in_=oacc[j][:])
            nc.sync.dma_start(out=out_d[j], in_=st[:])

    return nc


def _get_nc():
    if "nc" not in _cache:
        nc = _build_nc()
        if not nc.is_finalized():
            # Runs Bacc.compile() legalization (wait splitting, reg alloc).
            nc.finalize()
        _cache["nc"] = nc
    return _cache["nc"]


def _host_prep(queries, keys, values):
    """Cast to fp16 and pre-arrange into the SBUF layouts (see _build_nc)."""
    k16 = np.asarray(keys, dtype=np.float16)
    v16 = np.asarray(values, dtype=np.float16)
    q16 = np.asarray(queries, dtype=np.float16)

    # kt: [8,4096,64] -> [8,64,4096] -> [4,128,4096] -> [128, 4*4096]
    kt = np.ascontiguousarray(
        k16.transpose(0, 2, 1).reshape(NPAIR, 128, S).transpose(1, 0, 2).reshape(128, NPAIR * S)
    )
    # vv: [8,4096,64] -> [8,32,128,64] -> [128,8,32,64] -> [128, 16384]
    vv = np.ascontiguousarray(
        v16.reshape(B, NKT, KT, D).transpose(2, 0, 1, 3).reshape(128, B * NKT * D)
    )

    qts = []
    for c in range(NCORES):
        qc = q16[:, c * QBLK : (c + 1) * QBLK, :]  # [8, 512, 64]
        qt = np.ascontiguousarray(
            qc.transpose(0, 2, 1).reshape(NPAIR, 128, QBLK).transpose(1, 0, 2).reshape(128, NPAIR * QBLK)
        )
        qts.append(qt)
    return qts, kt, vv


def kernel(queries, keys, values):
    global LAST_RESULT
    from concourse.bass_utils import run_bass_kernel_spmd

    queries = np.asarray(queries, dtype=np.float32)
    keys = np.asarray(keys, dtype=np.float32)
    values = np.asarray(values, dtype=np.float32)

    nc = _get_nc()
    qts, kt, vv = _host_prep(queries, keys, values)
    in_maps = [{"qt": qts[c], "kt": kt, "vv": vv} for c in range(NCORES)]

    res = run_bass_kernel_spmd(
        nc,
        in_maps,
        list(range(NCORES)),
        trace=TRACE,
        **TRACE_KWARGS,
    )
    LAST_RESULT = res

    out = np.empty((B, S, D), dtype=np.float32)
    for c in range(NCORES):
        o = res.results[c]["out"]  # [4, 128, 512] = [j, (b%2)*64+d, q]
        out[:, c * QBLK : (c + 1) * QBLK, :] = (
            o.reshape(B, D, QBLK).transpose(0, 2, 1)
        )
    return out
